# revision 13
# baseline (speedup 1.0000x reference)
"""Trainium2 Bass kernel for GQA attention (B=2, S=2048, D=2048, 16 q-heads /
4 kv-heads, HD=128) with per-head QK RMSNorm + RoPE + causal softmax + output
projection.

Sharding: 8 cores = (batch b in {0,1}) x (kv-group g in {0..3}). Each core
computes its batch's 4 q-heads + 1 kv-head and a partial output through the
row-sharded Wo; the host sums the 4 partials per batch.

v4: transposed-scores (S^T) attention with fully interleaved phases.
Scores are computed as [k, q] tiles (kT-stationary), exp moves them
PSUM->SBUF (no transposes of probs), PV is V-stationary giving attention
output directly in [d, q] layout. Softmax denominators come from an
all-ones-stationary matmul over the same probsT stream (pre-broadcast
across partitions). RMSNorm scales fold into the phase-1 transposes via
diag(rs) companions. qT/kT/vv/aoT are split per 512-superblock so the
tile dependency tracker lets QKV (p1), attention (p2) and out-projection
(p3) units interleave in one long pipeline.
"""
import numpy as np

import concourse.bass as bass  # noqa: F401
import concourse.mybir as mybir
import concourse.tile as tile
from concourse import bacc
from concourse.bass_utils import run_bass_kernel_spmd

F32 = mybir.dt.float32
F16 = mybir.dt.float16
AF = mybir.ActivationFunctionType
OP = mybir.AluOpType

B, S, D = 2, 2048, 2048
NH, NKV, HD = 16, 4, 128
REP = NH // NKV
EPS = 1e-6
NEG = -1.0e30
EXPB = -5.0  # exp bias: cancels in softmax, keeps exp() in fp16 range


def build(s=S):
    """Build + compile the per-core SPMD program (identical on all 8 cores)."""
    sc = s // 128          # s-chunks
    kc = D // 128          # contraction chunks
    nsb = sc // 4          # 512-wide superblocks
    nc = bacc.Bacc("TRN2", target_bir_lowering=False, debug=False, num_devices=8)

    xT_d = nc.dram_tensor("xT", [D, s], F16, kind="ExternalInput")
    wqkv_d = nc.dram_tensor("wqkv", [D, 768], F16, kind="ExternalInput")
    wo_d = nc.dram_tensor("wo", [512, D], F16, kind="ExternalInput")
    ropes_d = nc.dram_tensor("ropes", [s, 4 * HD], F32, kind="ExternalInput")
    mask_d = nc.dram_tensor("maskbT", [128, 128], F32, kind="ExternalInput")
    iden16_d = nc.dram_tensor("ident16", [128, 128], F16, kind="ExternalInput")
    out_d = nc.dram_tensor("outp", [s, D], F16, kind="ExternalOutput")

    with tile.TileContext(nc) as tc:
        with (
            tc.tile_pool(name="pers", bufs=1) as pers,
            tc.tile_pool(name="ob", bufs=3) as ob,
            tc.tile_pool(name="xp", bufs=4) as xp,
            tc.tile_pool(name="cp", bufs=4) as cp,
            tc.tile_pool(name="st", bufs=3) as st,
            tc.tile_pool(name="pp", bufs=12) as pp,
            tc.tile_pool(name="sm", bufs=3) as sm,
            tc.tile_pool(name="psA", bufs=2, space="PSUM") as psA,
            tc.tile_pool(name="psX", bufs=2, space="PSUM") as psX,
        ):
            qTs = [pers.tile([128, REP, 512], F16, tag=f"qT{q}", name=f"qT{q}") for q in range(nsb)]
            kTs = [pers.tile([128, 512], F16, tag=f"kT{q}", name=f"kT{q}") for q in range(nsb)]
            vvs = [pers.tile([128, 4, HD], F16, tag=f"vv{q}", name=f"vv{q}") for q in range(nsb)]
            aoTs = [pers.tile([128, REP, 512], F16, tag=f"aoT{q}", name=f"aoT{q}") for q in range(nsb)]
            wqkv_t = pers.tile([128, kc, 768], F16, tag="wqkv")
            wo_t = pers.tile([128, REP, D], F16, tag="wo")
            mask_t = pers.tile([128, 128], F32, tag="maskbT")
            iden16_t = pers.tile([128, 128], F16, tag="ident16")
            eps_t = pers.tile([128, 1], F32, tag="eps")
            nc.vector.memset(eps_t[:], EPS)
            expb_t = pers.tile([128, 1], F32, tag="expb")
            nc.vector.memset(expb_t[:], EXPB)
            ones_t = pers.tile([128, 128], F16, tag="ones")
            nc.vector.memset(ones_t[:], 1.0)

            wqkv_r = wqkv_d.rearrange("(dk ki) e -> ki dk e", ki=128)
            ropes_r = ropes_d.rearrange("(m si) h -> si m h", si=128)
            xT_r = xT_d.rearrange("(dk ki) t -> ki dk t", ki=128)

            for k in range(kc):
                nc.sync.dma_start(out=wqkv_t[:, k], in_=wqkv_r[:, k])
            nc.sync.dma_start(out=mask_t[:], in_=mask_d[:, :])
            nc.sync.dma_start(out=iden16_t[:], in_=iden16_d[:, :])
            nc.sync.dma_start(
                out=wo_t[:], in_=wo_d.rearrange("(e ki) d -> ki e d", ki=128),
            )

            # ---------------- Phase-1 unit: QKV + RMSNorm + RoPE ------------
            pend = []

            def emit_transposes():
                qn_, kn_, dg_, mm = pend.pop()
                pt = psX.tile([128, 1024], F32, tag="psX")
                for e in range(REP):
                    nc.tensor.matmul(
                        pt[:, e * 128:(e + 1) * 128],
                        qn_[:, e * 128:(e + 1) * 128],
                        dg_[:, e * 128:(e + 1) * 128],
                        start=True, stop=True,
                    )
                nc.tensor.matmul(
                    pt[:, 512:640], kn_[:], dg_[:, 512:640],
                    start=True, stop=True,
                )
                g, l = mm // 4, mm % 4
                for e in range(REP):
                    nc.vector.tensor_copy(
                        out=qTs[g][:, e, l * 128:(l + 1) * 128],
                        in_=pt[:, e * 128:(e + 1) * 128],
                    )
                nc.vector.tensor_copy(
                    out=kTs[g][:, l * 128:(l + 1) * 128], in_=pt[:, 512:640],
                )

            def p1(m):
                xt = xp.tile([128, kc, 128], F16, tag="xt")
                nc.gpsimd.dma_start(out=xt[:], in_=xT_r[:, :, m * 128:(m + 1) * 128])
                cst = cp.tile([128, 512], F32, tag="cst")
                nc.gpsimd.dma_start(out=cst[:], in_=ropes_r[:, m])
                cq = cst[:, 0:128]
                sq_ = cst[:, 128:256]
                ck = cst[:, 256:384]
                sk_ = cst[:, 384:512]

                pqkv = psA.tile([128, 1024], F32, tag="psA")
                pq = pqkv[:, 0:512]
                pkv = pqkv[:, 512:768]
                for k in range(kc):
                    nc.tensor.matmul(
                        pq, xt[:, k], wqkv_t[:, k, 0:512],
                        start=(k == 0), stop=(k == kc - 1),
                    )
                for k in range(kc):
                    nc.tensor.matmul(
                        pkv, xt[:, k], wqkv_t[:, k, 512:768],
                        start=(k == 0), stop=(k == kc - 1),
                    )
                if pend:
                    emit_transposes()

                # ---- RMSNorm stats: per-head sum of squares via ACT ----
                ss = st.tile([128, 16], F32, tag="ss")
                sqd = st.tile([128, 128], F32, tag="sqd")
                for e in range(REP):
                    nc.scalar.activation(
                        sqd[:], pq[:, e * 128:(e + 1) * 128], AF.Square,
                        accum_out=ss[:, e:e + 1],
                    )
                nc.scalar.activation(
                    sqd[:], pkv[:, 0:128], AF.Square, accum_out=ss[:, 4:5],
                )
                nc.scalar.activation(
                    ss[:, 8:13], ss[:, 0:5], AF.Sqrt,
                    bias=eps_t[:], scale=1.0 / HD,
                )
                rs = st.tile([128, 8], F32, tag="rs")
                nc.vector.reciprocal(rs[:, 0:5], ss[:, 8:13])
                # diag(rs) companions for the normalizing transposes
                dg = st.tile([128, 640], F16, tag="dg")
                for e in range(5):
                    nc.vector.tensor_scalar_mul(
                        dg[:, e * 128:(e + 1) * 128], iden16_t[:],
                        rs[:, e:e + 1],
                    )

                # ---- RoPE on raw q (norm scale folded into transpose) ----
                pq3 = pq.rearrange("p (h d) -> p h d", d=128)
                qn = st.tile([128, 512], F16, tag="qn")
                qn3 = qn[:].rearrange("p (h d) -> p h d", d=128)
                ra = st.tile([128, REP, 128], F32, tag="ra")
                nc.vector.tensor_mul(
                    ra[:], pq3,
                    cq.rearrange("p (o d) -> p o d", o=1).broadcast_to(
                        [128, REP, 128]),
                )
                rb = st.tile([128, REP, 128], F32, tag="rb")
                nc.vector.tensor_mul(
                    rb[:, :, 0:64], pq3[:, :, 64:128],
                    sq_[:, 0:64].rearrange("p (o d) -> p o d", o=1).broadcast_to(
                        [128, REP, 64]),
                )
                nc.vector.tensor_mul(
                    rb[:, :, 64:128], pq3[:, :, 0:64],
                    sq_[:, 64:128].rearrange("p (o d) -> p o d", o=1).broadcast_to(
                        [128, REP, 64]),
                )
                nc.vector.tensor_add(qn3, ra[:], rb[:])

                # ---- k head rope ----
                kn = st.tile([128, 128], F16, tag="kn")
                rak = st.tile([128, 128], F32, tag="rak")
                nc.vector.tensor_mul(rak[:], pkv[:, 0:128], ck)
                rbk = st.tile([128, 128], F32, tag="rbk")
                nc.vector.tensor_mul(rbk[:, 0:64], pkv[:, 64:128], sk_[:, 0:64])
                nc.vector.tensor_mul(rbk[:, 64:128], pkv[:, 0:64], sk_[:, 64:128])
                nc.vector.tensor_add(kn[:], rak[:], rbk[:])

                nc.vector.tensor_copy(out=vvs[m // 4][:, m % 4, :],
                                      in_=pkv[:, 128:256])
                pend.append((qn, kn, dg, m))

            # ------------- Phase-2 unit: attention superblock ---------------
            def p2(Q, h):
                nj = 4 * Q + 4
                pairs = []
                for p in range(nj // 2):
                    ps = psX.tile([128, 1024], F32, tag="psX")
                    pb = pp.tile([128, 2, 512], F16, tag="pb")
                    pairs.append(pb)
                    for t in (0, 1):
                        j = 2 * p + t
                        b = j - 4 * Q
                        c0 = max(0, b * 128)
                        nc.tensor.matmul(
                            ps[:, t * 512 + c0:(t + 1) * 512],
                            kTs[j // 4][:, (j % 4) * 128:(j % 4 + 1) * 128],
                            qTs[Q][:, h, c0:512],
                            start=True, stop=True,
                        )
                        if b >= 0:
                            nc.vector.tensor_add(
                                ps[:, t * 512 + c0:t * 512 + c0 + 128],
                                ps[:, t * 512 + c0:t * 512 + c0 + 128],
                                mask_t[:],
                            )
                    nc.scalar.activation(
                        pb[:], ps[:, :], AF.Exp, bias=expb_t[:],
                    )
                pod = psA.tile([128, 1024], F32, tag="psA")
                po = pod[:, 0:512]
                pd = pod[:, 512:1024]
                for j in range(nj):
                    c0 = max(0, (j - 4 * Q) * 128)
                    pbj = pairs[j // 2][:, j % 2, c0:]
                    nc.tensor.matmul(
                        po[:, c0:], vvs[j // 4][:, j % 4, :], pbj,
                        start=(j == 0), stop=(j == nj - 1),
                        skip_group_check=True,
                    )
                    nc.tensor.matmul(
                        pd[:, c0:], ones_t[:], pbj,
                        start=(j == 0), stop=(j == nj - 1),
                        skip_group_check=True,
                    )
                rec = sm.tile([128, 512], F32, tag="rec")
                nc.vector.reciprocal_approx_fast(out=rec[:], in_=pd)
                nc.vector.tensor_mul(aoTs[Q][:, h, :], po, rec[:])

            # ------------- Phase-3 unit: out-projection chunk ---------------
            def p3(m):
                ot = ob.tile([128, D], F16, tag="ot")
                for nn in range(2):
                    pon = psA.tile([128, 1024], F32, tag="psA")
                    for half in range(2):
                        n = nn * 2 + half
                        for e in range(REP):
                            nc.tensor.matmul(
                                pon[:, half * 512:(half + 1) * 512],
                                aoTs[m // 4][:, e, (m % 4) * 128:(m % 4 + 1) * 128],
                                wo_t[:, e, n * 512:(n + 1) * 512],
                                start=(e == 0), stop=(e == REP - 1),
                                skip_group_check=True,
                            )
                    nc.vector.tensor_copy(
                        out=ot[:, nn * 1024:nn * 1024 + 512], in_=pon[:, 0:512],
                    )
                    nc.scalar.copy(
                        out=ot[:, nn * 1024 + 512:(nn + 1) * 1024],
                        in_=pon[:, 512:1024],
                    )
                eng = nc.gpsimd
                eng.dma_start(out=out_d[m * 128:(m + 1) * 128, :], in_=ot[:])

            # --------------------- interleaved schedule ---------------------
            sched = []
            p2q = [(Q, h) for Q in range(nsb) for h in range(REP)]
            # p1 m=0..4 head start; then round-robin p1/p2/p3 respecting
            # readiness: p2(Q,*) once transposes cover m=4Q+3; p3(m) after
            # p2(m//4, h=3).
            for m in range(5):
                sched.append(("p1", m))
            np1, np2, np3 = 5, 0, 0
            tdone = 4  # transposes emitted for m < tdone
            t_emitted = False
            while np2 < len(p2q) or np3 < sc:
                if np2 < len(p2q) and p2q[np2][0] * 4 + 4 <= tdone:
                    sched.append(("p2", p2q[np2]))
                    np2 += 1
                if np3 < sc and np3 // 4 * 4 + 4 <= np2:
                    sched.append(("p3", np3))
                    np3 += 1
                if np1 < sc:
                    sched.append(("p1", np1))
                    np1 += 1
                    tdone = np1 - 1
                elif not t_emitted:
                    sched.append(("pT", None))
                    t_emitted = True
                    tdone = sc

            for kind, arg in sched:
                if kind == "p1":
                    p1(arg)
                elif kind == "pT":
                    emit_transposes()
                elif kind == "p2":
                    p2(*arg)
                else:
                    p3(arg)

    nc.compile()
    return nc


def make_in_maps(x, cos, sin, Wq, Wk, Wv, Wo, q_norm_w, k_norm_w):
    qsc = (q_norm_w / np.sqrt(HD)).astype(np.float32)
    ksc = k_norm_w.astype(np.float32)

    def rope_consts(w):
        cw = (cos * w[None, :]).astype(np.float32)
        sw = np.empty_like(cw)
        sw[:, :64] = -sin[:, :64] * w[None, 64:]
        sw[:, 64:] = sin[:, 64:] * w[None, :64]
        return cw, sw

    cwq, swq = rope_consts(qsc)
    cwk, swk = rope_consts(ksc)
    ropes = np.ascontiguousarray(np.concatenate([cwq, swq, cwk, swk], axis=1))
    r = np.arange(128)
    # transposed causal mask: rows = k, cols = q; masked where q < k
    maskbT = np.where(r[None, :] < r[:, None], NEG, 0.0).astype(np.float32)
    ident16 = np.eye(128, dtype=np.float16)

    in_maps = []
    for c in range(8):
        b, g = c // 4, c % 4
        xT = np.ascontiguousarray(x[b].T.astype(np.float16))
        wqkv = np.ascontiguousarray(
            np.concatenate(
                [
                    Wq[:, g * 512:(g + 1) * 512],
                    Wk[:, g * 128:(g + 1) * 128],
                    Wv[:, g * 128:(g + 1) * 128],
                ],
                axis=1,
            ).astype(np.float16)
        )
        wo = np.ascontiguousarray(Wo[g * 512:(g + 1) * 512, :].astype(np.float16))
        in_maps.append(
            dict(
                xT=xT, wqkv=wqkv, wo=wo, ropes=ropes,
                maskbT=maskbT, ident16=ident16,
            )
        )
    return in_maps


_cached = None


def kernel(x, cos, sin, Wq, Wk, Wv, Wo, q_norm_w, k_norm_w):
    global _cached
    x = np.asarray(x, np.float32)
    cos = np.asarray(cos, np.float32)
    sin = np.asarray(sin, np.float32)
    in_maps = make_in_maps(
        x, cos, sin,
        np.asarray(Wq, np.float32), np.asarray(Wk, np.float32),
        np.asarray(Wv, np.float32), np.asarray(Wo, np.float32),
        np.asarray(q_norm_w, np.float32), np.asarray(k_norm_w, np.float32),
    )
    if _cached is None:
        _cached = build()
    res = run_bass_kernel_spmd(_cached, in_maps, core_ids=list(range(8)))
    out = np.zeros((B, S, D), np.float64)
    for c in range(8):
        out[c // 4] += res.results[c]["outp"].astype(np.float64)
    return out.astype(np.float32)


# revision 15
# speedup vs baseline: 1.0020x; 1.0020x over previous
"""Trainium2 Bass kernel for GQA attention (B=2, S=2048, D=2048, 16 q-heads /
4 kv-heads, HD=128) with per-head QK RMSNorm + RoPE + causal softmax + output
projection.

Sharding: 8 cores = (batch b in {0,1}) x (kv-group g in {0..3}). Each core
computes its batch's 4 q-heads + 1 kv-head and a partial output through the
row-sharded Wo; the host sums the 4 partials per batch.

v4: transposed-scores (S^T) attention with fully interleaved phases.
Scores are computed as [k, q] tiles (kT-stationary), exp moves them
PSUM->SBUF (no transposes of probs), PV is V-stationary giving attention
output directly in [d, q] layout. Softmax denominators come from an
all-ones-stationary matmul over the same probsT stream (pre-broadcast
across partitions). RMSNorm scales fold into the phase-1 transposes via
diag(rs) companions. qT/kT/vv/aoT are split per 512-superblock so the
tile dependency tracker lets QKV (p1), attention (p2) and out-projection
(p3) units interleave in one long pipeline.
"""
import numpy as np

import concourse.bass as bass  # noqa: F401
import concourse.mybir as mybir
import concourse.tile as tile
from concourse import bacc
from concourse.bass_utils import run_bass_kernel_spmd

F32 = mybir.dt.float32
F16 = mybir.dt.float16
AF = mybir.ActivationFunctionType
OP = mybir.AluOpType

B, S, D = 2, 2048, 2048
NH, NKV, HD = 16, 4, 128
REP = NH // NKV
EPS = 1e-6
NEG = -1.0e30
EXPB = -5.0  # exp bias: cancels in softmax, keeps exp() in fp16 range


def build(s=S):
    """Build + compile the per-core SPMD program (identical on all 8 cores)."""
    sc = s // 128          # s-chunks
    kc = D // 128          # contraction chunks
    nsb = sc // 4          # 512-wide superblocks
    nc = bacc.Bacc("TRN2", target_bir_lowering=False, debug=False, num_devices=8)

    xT_d = nc.dram_tensor("xT", [D, s], F16, kind="ExternalInput")
    wqkv_d = nc.dram_tensor("wqkv", [D, 768], F16, kind="ExternalInput")
    wo_d = nc.dram_tensor("wo", [512, D], F16, kind="ExternalInput")
    ropes_d = nc.dram_tensor("ropes", [s, 4 * HD], F32, kind="ExternalInput")
    mask_d = nc.dram_tensor("maskbT", [128, 128], F32, kind="ExternalInput")
    iden16_d = nc.dram_tensor("ident16", [128, 128], F16, kind="ExternalInput")
    out_d = nc.dram_tensor("outp", [s, D], F16, kind="ExternalOutput")

    with tile.TileContext(nc) as tc:
        with (
            tc.tile_pool(name="pers", bufs=1) as pers,
            tc.tile_pool(name="ob", bufs=3) as ob,
            tc.tile_pool(name="xp", bufs=4) as xp,
            tc.tile_pool(name="cp", bufs=4) as cp,
            tc.tile_pool(name="st", bufs=3) as st,
            tc.tile_pool(name="pp", bufs=12) as pp,
            tc.tile_pool(name="sm", bufs=3) as sm,
            tc.tile_pool(name="psA", bufs=2, space="PSUM") as psA,
            tc.tile_pool(name="psX", bufs=2, space="PSUM") as psX,
        ):
            qTs = [pers.tile([128, REP, 512], F16, tag=f"qT{q}", name=f"qT{q}") for q in range(nsb)]
            kTs = [pers.tile([128, 512], F16, tag=f"kT{q}", name=f"kT{q}") for q in range(nsb)]
            vvs = [pers.tile([128, 4, HD], F16, tag=f"vv{q}", name=f"vv{q}") for q in range(nsb)]
            aoTs = [pers.tile([128, REP, 512], F16, tag=f"aoT{q}", name=f"aoT{q}") for q in range(nsb)]
            wqkv_t = pers.tile([128, kc, 768], F16, tag="wqkv")
            wo_t = pers.tile([128, REP, D], F16, tag="wo")
            mask_t = pers.tile([128, 128], F32, tag="maskbT")
            iden16_t = pers.tile([128, 128], F16, tag="ident16")
            eps_t = pers.tile([128, 1], F32, tag="eps")
            nc.vector.memset(eps_t[:], EPS)
            expb_t = pers.tile([128, 1], F32, tag="expb")
            nc.vector.memset(expb_t[:], EXPB)
            ones_t = pers.tile([128, 128], F16, tag="ones")
            nc.vector.memset(ones_t[:], 1.0)

            wqkv_r = wqkv_d.rearrange("(dk ki) e -> ki dk e", ki=128)
            ropes_r = ropes_d.rearrange("(m si) h -> si m h", si=128)
            xT_r = xT_d.rearrange("(dk ki) t -> ki dk t", ki=128)

            for k in range(kc):
                nc.sync.dma_start(out=wqkv_t[:, k], in_=wqkv_r[:, k])
            nc.sync.dma_start(out=mask_t[:], in_=mask_d[:, :])
            nc.sync.dma_start(out=iden16_t[:], in_=iden16_d[:, :])
            nc.sync.dma_start(
                out=wo_t[:], in_=wo_d.rearrange("(e ki) d -> ki e d", ki=128),
            )

            # ---------------- Phase-1 unit: QKV + RMSNorm + RoPE ------------
            pend = []

            def emit_transposes():
                qn_, kn_, dg_, mm = pend.pop()
                pt = psX.tile([128, 1024], F32, tag="psX")
                for e in range(REP):
                    nc.tensor.matmul(
                        pt[:, e * 128:(e + 1) * 128],
                        qn_[:, e * 128:(e + 1) * 128],
                        dg_[:, e * 128:(e + 1) * 128],
                        start=True, stop=True,
                    )
                nc.tensor.matmul(
                    pt[:, 512:640], kn_[:], dg_[:, 512:640],
                    start=True, stop=True,
                )
                g, l = mm // 4, mm % 4
                for e in range(REP):
                    nc.vector.tensor_copy(
                        out=qTs[g][:, e, l * 128:(l + 1) * 128],
                        in_=pt[:, e * 128:(e + 1) * 128],
                    )
                nc.vector.tensor_copy(
                    out=kTs[g][:, l * 128:(l + 1) * 128], in_=pt[:, 512:640],
                )

            def p1(m):
                xt = xp.tile([128, kc, 128], F16, tag="xt")
                nc.gpsimd.dma_start(out=xt[:], in_=xT_r[:, :, m * 128:(m + 1) * 128])
                cst = cp.tile([128, 512], F32, tag="cst")
                nc.gpsimd.dma_start(out=cst[:], in_=ropes_r[:, m])
                cq = cst[:, 0:128]
                sq_ = cst[:, 128:256]
                ck = cst[:, 256:384]
                sk_ = cst[:, 384:512]

                pqkv = psA.tile([128, 1024], F32, tag="psA")
                pq = pqkv[:, 0:512]
                pkv = pqkv[:, 512:768]
                for k in range(kc):
                    nc.tensor.matmul(
                        pq, xt[:, k], wqkv_t[:, k, 0:512],
                        start=(k == 0), stop=(k == kc - 1),
                    )
                for k in range(kc):
                    nc.tensor.matmul(
                        pkv, xt[:, k], wqkv_t[:, k, 512:768],
                        start=(k == 0), stop=(k == kc - 1),
                    )
                if pend:
                    emit_transposes()

                # ---- RMSNorm stats: per-head sum of squares via ACT ----
                ss = st.tile([128, 16], F32, tag="ss")
                sqd = st.tile([128, 128], F32, tag="sqd")
                for e in range(REP):
                    nc.scalar.activation(
                        sqd[:], pq[:, e * 128:(e + 1) * 128], AF.Square,
                        accum_out=ss[:, e:e + 1],
                    )
                nc.scalar.activation(
                    sqd[:], pkv[:, 0:128], AF.Square, accum_out=ss[:, 4:5],
                )
                nc.scalar.activation(
                    ss[:, 8:13], ss[:, 0:5], AF.Sqrt,
                    bias=eps_t[:], scale=1.0 / HD,
                )
                rs = st.tile([128, 8], F32, tag="rs")
                nc.vector.reciprocal(rs[:, 0:5], ss[:, 8:13])
                # diag(rs) companions for the normalizing transposes
                dg = st.tile([128, 640], F16, tag="dg")
                for e in range(5):
                    nc.vector.tensor_scalar_mul(
                        dg[:, e * 128:(e + 1) * 128], iden16_t[:],
                        rs[:, e:e + 1],
                    )

                # ---- RoPE on raw q (norm scale folded into transpose) ----
                pq3 = pq.rearrange("p (h d) -> p h d", d=128)
                qn = st.tile([128, 512], F16, tag="qn")
                qn3 = qn[:].rearrange("p (h d) -> p h d", d=128)
                ra = st.tile([128, REP, 128], F32, tag="ra")
                nc.vector.tensor_mul(
                    ra[:], pq3,
                    cq.rearrange("p (o d) -> p o d", o=1).broadcast_to(
                        [128, REP, 128]),
                )
                rb = st.tile([128, REP, 128], F32, tag="rb")
                nc.vector.tensor_mul(
                    rb[:, :, 0:64], pq3[:, :, 64:128],
                    sq_[:, 0:64].rearrange("p (o d) -> p o d", o=1).broadcast_to(
                        [128, REP, 64]),
                )
                nc.vector.tensor_mul(
                    rb[:, :, 64:128], pq3[:, :, 0:64],
                    sq_[:, 64:128].rearrange("p (o d) -> p o d", o=1).broadcast_to(
                        [128, REP, 64]),
                )
                nc.vector.tensor_add(qn3, ra[:], rb[:])

                # ---- k head rope ----
                kn = st.tile([128, 128], F16, tag="kn")
                rak = st.tile([128, 128], F32, tag="rak")
                nc.vector.tensor_mul(rak[:], pkv[:, 0:128], ck)
                rbk = st.tile([128, 128], F32, tag="rbk")
                nc.vector.tensor_mul(rbk[:, 0:64], pkv[:, 64:128], sk_[:, 0:64])
                nc.vector.tensor_mul(rbk[:, 64:128], pkv[:, 0:64], sk_[:, 64:128])
                nc.vector.tensor_add(kn[:], rak[:], rbk[:])

                nc.vector.tensor_copy(out=vvs[m // 4][:, m % 4, :],
                                      in_=pkv[:, 128:256])
                pend.append((qn, kn, dg, m))

            # ------------- Phase-2 unit: attention superblock ---------------
            def p2(Q, h):
                nj = 4 * Q + 4
                pairs = []
                for p in range(nj // 2):
                    ps = psX.tile([128, 1024], F32, tag="psX")
                    pb = pp.tile([128, 2, 512], F16, tag="pb")
                    pairs.append(pb)
                    for t in (0, 1):
                        j = 2 * p + t
                        b = j - 4 * Q
                        c0 = max(0, b * 128)
                        nc.tensor.matmul(
                            ps[:, t * 512 + c0:(t + 1) * 512],
                            kTs[j // 4][:, (j % 4) * 128:(j % 4 + 1) * 128],
                            qTs[Q][:, h, c0:512],
                            start=True, stop=True,
                        )
                        if b >= 0:
                            nc.vector.tensor_add(
                                ps[:, t * 512 + c0:t * 512 + c0 + 128],
                                ps[:, t * 512 + c0:t * 512 + c0 + 128],
                                mask_t[:],
                            )
                    nc.scalar.activation(
                        pb[:], ps[:, :], AF.Exp, bias=expb_t[:],
                    )
                pod = psA.tile([128, 1024], F32, tag="psA")
                po = pod[:, 0:512]
                pd = pod[:, 512:1024]
                for j in range(nj):
                    c0 = max(0, (j - 4 * Q) * 128)
                    pbj = pairs[j // 2][:, j % 2, c0:]
                    nc.tensor.matmul(
                        po[:, c0:], vvs[j // 4][:, j % 4, :], pbj,
                        start=(j == 0), stop=(j == nj - 1),
                        skip_group_check=True,
                    )
                    nc.tensor.matmul(
                        pd[:, c0:], ones_t[:], pbj,
                        start=(j == 0), stop=(j == nj - 1),
                        skip_group_check=True,
                    )
                rec = sm.tile([128, 512], F32, tag="rec")
                nc.vector.reciprocal_approx_fast(out=rec[:], in_=pd)
                nc.vector.tensor_mul(aoTs[Q][:, h, :], po, rec[:])

            # ------------- Phase-3 unit: out-projection chunk ---------------
            def p3(m):
                ot = ob.tile([128, D], F16, tag="ot")
                for nn in range(2):
                    pon = psA.tile([128, 1024], F32, tag="psA")
                    for half in range(2):
                        n = nn * 2 + half
                        for e in range(REP):
                            nc.tensor.matmul(
                                pon[:, half * 512:(half + 1) * 512],
                                aoTs[m // 4][:, e, (m % 4) * 128:(m % 4 + 1) * 128],
                                wo_t[:, e, n * 512:(n + 1) * 512],
                                start=(e == 0), stop=(e == REP - 1),
                                skip_group_check=True,
                            )
                    nc.vector.tensor_copy(
                        out=ot[:, nn * 1024:nn * 1024 + 512], in_=pon[:, 0:512],
                    )
                    nc.scalar.copy(
                        out=ot[:, nn * 1024 + 512:(nn + 1) * 1024],
                        in_=pon[:, 512:1024],
                    )
                # sync-engine trigger: its stream has nothing latency-critical
                # behind the initial weight loads, so the copy-wait is harmless
                nc.sync.dma_start(out=out_d[m * 128:(m + 1) * 128, :], in_=ot[:])

            # --------------------- interleaved schedule ---------------------
            sched = []
            p2q = [(Q, h) for Q in range(nsb) for h in range(REP)]
            # p1 m=0..4 head start; then round-robin p1/p2/p3 respecting
            # readiness: p2(Q,*) once transposes cover m=4Q+3; p3(m) after
            # p2(m//4, h=3).
            for m in range(5):
                sched.append(("p1", m))
            np1, np2, np3 = 5, 0, 0
            tdone = 4  # transposes emitted for m < tdone
            t_emitted = False
            while np2 < len(p2q) or np3 < sc:
                if np2 < len(p2q) and p2q[np2][0] * 4 + 4 <= tdone:
                    sched.append(("p2", p2q[np2]))
                    np2 += 1
                if np3 < sc and np3 // 4 * 4 + 4 <= np2:
                    sched.append(("p3", np3))
                    np3 += 1
                if np1 < sc:
                    sched.append(("p1", np1))
                    np1 += 1
                    tdone = np1 - 1
                elif not t_emitted:
                    sched.append(("pT", None))
                    t_emitted = True
                    tdone = sc

            for kind, arg in sched:
                if kind == "p1":
                    p1(arg)
                elif kind == "pT":
                    emit_transposes()
                elif kind == "p2":
                    p2(*arg)
                else:
                    p3(arg)

    nc.compile()
    return nc


def make_in_maps(x, cos, sin, Wq, Wk, Wv, Wo, q_norm_w, k_norm_w):
    qsc = (q_norm_w / np.sqrt(HD)).astype(np.float32)
    ksc = k_norm_w.astype(np.float32)

    def rope_consts(w):
        cw = (cos * w[None, :]).astype(np.float32)
        sw = np.empty_like(cw)
        sw[:, :64] = -sin[:, :64] * w[None, 64:]
        sw[:, 64:] = sin[:, 64:] * w[None, :64]
        return cw, sw

    cwq, swq = rope_consts(qsc)
    cwk, swk = rope_consts(ksc)
    ropes = np.ascontiguousarray(np.concatenate([cwq, swq, cwk, swk], axis=1))
    r = np.arange(128)
    # transposed causal mask: rows = k, cols = q; masked where q < k
    maskbT = np.where(r[None, :] < r[:, None], NEG, 0.0).astype(np.float32)
    ident16 = np.eye(128, dtype=np.float16)

    in_maps = []
    for c in range(8):
        b, g = c // 4, c % 4
        xT = np.ascontiguousarray(x[b].T.astype(np.float16))
        wqkv = np.ascontiguousarray(
            np.concatenate(
                [
                    Wq[:, g * 512:(g + 1) * 512],
                    Wk[:, g * 128:(g + 1) * 128],
                    Wv[:, g * 128:(g + 1) * 128],
                ],
                axis=1,
            ).astype(np.float16)
        )
        wo = np.ascontiguousarray(Wo[g * 512:(g + 1) * 512, :].astype(np.float16))
        in_maps.append(
            dict(
                xT=xT, wqkv=wqkv, wo=wo, ropes=ropes,
                maskbT=maskbT, ident16=ident16,
            )
        )
    return in_maps


_cached = None


def kernel(x, cos, sin, Wq, Wk, Wv, Wo, q_norm_w, k_norm_w):
    global _cached
    x = np.asarray(x, np.float32)
    cos = np.asarray(cos, np.float32)
    sin = np.asarray(sin, np.float32)
    in_maps = make_in_maps(
        x, cos, sin,
        np.asarray(Wq, np.float32), np.asarray(Wk, np.float32),
        np.asarray(Wv, np.float32), np.asarray(Wo, np.float32),
        np.asarray(q_norm_w, np.float32), np.asarray(k_norm_w, np.float32),
    )
    if _cached is None:
        _cached = build()
    res = run_bass_kernel_spmd(_cached, in_maps, core_ids=list(range(8)))
    out = np.zeros((B, S, D), np.float64)
    for c in range(8):
        out[c // 4] += res.results[c]["outp"].astype(np.float64)
    return out.astype(np.float32)
